# revision 27
# baseline (speedup 1.0000x reference)
"""Trainium2 Bass kernel: causal self-attention (modded-nanogpt style),
tensor-parallel over heads across 8 NeuronCores with an on-device AllToAll
re-shard before the output projection.

Self-contained: hardcodes B=1, T=4096, D=1024, H=8, Hd=128, scale=0.12.

Per-core program (core = head), processed in 8 groups of four 128-row tiles:
  qkv_stage(g)   4x[ qkv matmuls (xT tile stationary), lambda-mix of v,
                     sum-of-squares stats (ACT Square + accum) ]
  attn_chunk(g-1) S^T = kT.T@qT per 128-key-block; ACT exp out of PSUM
                 (2^-12 folded into the bias keeps fp16 in range and cancels
                 in the normalize); denominator = DVE fp16 adds + one
                 ones-matmul; y^T += v.T @ expT
  norm_stage(g)  batched rsqrt via DVE integer magic + 2 Newton steps (no
                 ACT table loads), q/k normalize, RoPE, PE transposes
Then an AllToAll re-shards heads -> sequence and the output projection runs
over all heads for this core's 512 rows.
"""

import os
import sys

sys.path.insert(0, "/opt/trn_rl_repo")

from contextlib import ExitStack

import numpy as np

import concourse.bass as bass
import concourse.bacc as bacc
import concourse.mybir as mybir
import concourse.tile as tile
from concourse.bass_utils import run_bass_kernel_spmd
from concourse.masks import make_identity

N_CORES = 8
T = 4096
D = 1024
H = 8
HD = 128
ATTN_SCALE = 0.12
P = 128
TCH = 512
NT = T // P          # 32 t-tiles
NC_CH = T // TCH     # 8 chunks / tile groups
SHARD = T // N_CORES
QUARTER = HD // 4

F32 = mybir.dt.float32
I32 = mybir.dt.int32
_MODE = os.environ.get("KBASS_MM_DT", "f16")
MMD = {"f32r": mybir.dt.float32r, "f16": mybir.dt.float16,
       "f32": F32}[_MODE]
NP_MMD = {"f32r": np.float32, "f16": np.float16, "f32": np.float32}[_MODE]
# exp(s - 12*ln2) = 2^-12 * exp(s): keeps fp16 exp values and their fp16
# partial sums in range; the scaling cancels in the softmax normalize.
EXP_BIAS = -8.317766166719343 if _MODE == "f16" else 0.0
RSQRT_MAGIC = 0x5F3759DF

_cached = {}


def build_module():
    nc = bacc.Bacc("TRN2", target_bir_lowering=False, debug=False,
                   num_devices=N_CORES)

    x_t = nc.dram_tensor("x_t", [D, T], MMD, kind="ExternalInput")
    w_qkv = nc.dram_tensor("w_qkv", [D, 3 * HD], MMD, kind="ExternalInput")
    cos_t = nc.dram_tensor("cos_t", [T, QUARTER], F32, kind="ExternalInput")
    sin_t = nc.dram_tensor("sin_t", [T, QUARTER], F32, kind="ExternalInput")
    ve_h = nc.dram_tensor("ve_h", [T, HD], F32, kind="ExternalInput")
    lam = nc.dram_tensor("lam", [P, 2], F32, kind="ExternalInput")
    cpw = nc.dram_tensor("cpw", [D, D], MMD, kind="ExternalInput")
    y_shard = nc.dram_tensor("y_shard", [SHARD, D], F32, kind="ExternalOutput")

    with tile.TileContext(nc) as tc, nc.allow_low_precision(
            reason="reduced-precision matmul operands"), ExitStack() as ctx:
        const = ctx.enter_context(tc.tile_pool(name="const", bufs=1))
        wqkv_pool = ctx.enter_context(tc.tile_pool(name="wqkv", bufs=1))
        big = ctx.enter_context(tc.tile_pool(name="big", bufs=1))
        xt_pool = ctx.enter_context(tc.tile_pool(name="xt", bufs=5))
        cs_pool = ctx.enter_context(tc.tile_pool(name="cs", bufs=4))
        ve_pool = ctx.enter_context(tc.tile_pool(name="vein", bufs=3))
        scr_pool = ctx.enter_context(tc.tile_pool(name="scr", bufs=3))
        stat_pool = ctx.enter_context(tc.tile_pool(name="stat", bufs=3))
        tin_pool = ctx.enter_context(tc.tile_pool(name="tin", bufs=10))
        exp_pool = ctx.enter_context(tc.tile_pool(name="exp", bufs=5))
        acc_pool = ctx.enter_context(tc.tile_pool(name="acc", bufs=2))
        rro_pool = ctx.enter_context(tc.tile_pool(name="rro", bufs=2))
        cpw_pool = ctx.enter_context(tc.tile_pool(name="cpw", bufs=16))
        ps = ctx.enter_context(tc.tile_pool(name="ps", bufs=5, space="PSUM"))
        psy = ctx.enter_context(tc.tile_pool(name="psy", bufs=3, space="PSUM"))
        dram = ctx.enter_context(tc.tile_pool(name="dram", bufs=1,
                                              space="DRAM"))

        # ---- weights first so their DMAs lead the queues ----
        wqkv_sb = wqkv_pool.tile([P, D // P, 3 * HD], MMD)
        for k in range(D // P):
            eng = nc.scalar if k % 2 else nc.sync
            eng.dma_start(out=wqkv_sb[:, k, :],
                          in_=w_qkv.ap()[k * P:(k + 1) * P, :])
        lam_sb = const.tile([P, 2], F32)
        nc.scalar.dma_start(out=lam_sb[:], in_=lam.ap())

        # ---- constants ----
        ones_f = const.tile([P, 1], F32)
        nc.vector.memset(ones_f[:], 1.0)
        ones_col = const.tile([P, 1], MMD)
        nc.scalar.copy(ones_col[:], ones_f[:])
        ones_row_f = const.tile([1, P], F32)
        nc.vector.memset(ones_row_f[:], 1.0)
        ones_row = const.tile([1, P], MMD)
        nc.scalar.copy(ones_row[:], ones_row_f[:])
        expb_col = const.tile([P, 1], F32)
        nc.vector.memset(expb_col[:], EXP_BIAS)
        masks = []
        mk_f = const.tile([P, TCH], F32)
        for m in range(4):
            nc.vector.memset(mk_f[:], 1.0)
            nc.gpsimd.affine_select(
                out=mk_f[:], in_=mk_f[:],
                compare_op=mybir.AluOpType.is_ge, fill=0.0,
                base=-128 * m, channel_multiplier=-1, pattern=[[1, TCH]])
            mk = const.tile([P, TCH], MMD, name=f"mask{m}")
            nc.scalar.copy(mk[:], mk_f[:])
            masks.append(mk)
        ident_f = const.tile([P, P], F32)
        make_identity(nc, ident_f)
        ident = const.tile([P, P], MMD)
        nc.scalar.copy(ident[:], ident_f[:])

        # ---- persistent per-block tensors (separate tiles => precise deps)
        kT_t = [big.tile([P, P], MMD, name=f"kT{j}") for j in range(NT)]
        v_t = [big.tile([P, HD], MMD, name=f"v{j}") for j in range(NT)]
        qT_c = [big.tile([P, TCH], MMD, name=f"qT{c}") for c in range(NC_CH)]
        yT_c = [big.tile([P, TCH], MMD, name=f"yT{c}") for c in range(NC_CH)]

        cc_in = dram.tile([N_CORES * P * TCH], MMD)
        cc_out = dram.tile([N_CORES * P * TCH], MMD)
        cc_in_v = cc_in[:].rearrange("(j p f) -> j p f", j=N_CORES, p=P)
        cc_out_v = cc_out[:].rearrange("(j p f) -> j p f", j=N_CORES, p=P)

        xt_tiles = {}

        def ensure_xt(i):  # i even: tile pair (i, i+1)
            if i in xt_tiles or i >= NT:
                return
            xt = xt_pool.tile([P, D // P, 2 * P], MMD, tag="xt",
                              name=f"xt{i}")
            nc.sync.dma_start(
                out=xt[:],
                in_=x_t.ap().rearrange("(k p) t -> p k t", p=P)
                    [:, :, i * P:(i + 2) * P])
            xt_tiles[i] = xt

        pending_den = []

        def flush_den(n):
            while len(pending_den) > n:
                ps_y, ps_r, c = pending_den.pop(0)
                rrow = rro_pool.tile([1, TCH], F32)
                nc.vector.reciprocal(rrow[:], ps_r[:])
                rb_sb = rro_pool.tile([P, TCH], F32, tag="rb_sb")
                nc.gpsimd.partition_broadcast(rb_sb[:], rrow[:])
                nc.vector.tensor_mul(yT_c[c][:], ps_y[:], rb_sb[:])
                nc.gpsimd.dma_start(out=cc_in_v[c], in_=yT_c[c][:])

        def qkv_stage(g):
            t0g = 4 * g * P
            ve_g = ve_pool.tile([P, 4, HD], F32, tag="ve", name=f"ve{g}")
            nc.sync.dma_start(
                out=ve_g[:],
                in_=ve_h.ap().rearrange("(n p) e -> p n e", p=P)
                    [:, 4 * g:4 * g + 4, :])
            cos_g = cs_pool.tile([P, 4, QUARTER], F32, tag="cos",
                                 name=f"cos{g}")
            sin_g = cs_pool.tile([P, 4, QUARTER], F32, tag="sin",
                                 name=f"sin{g}")
            nc.sync.dma_start(
                out=cos_g[:],
                in_=cos_t.ap().rearrange("(n p) e -> p n e", p=P)
                    [:, 4 * g:4 * g + 4, :])
            nc.sync.dma_start(
                out=sin_g[:],
                in_=sin_t.ap().rearrange("(n p) e -> p n e", p=P)
                    [:, 4 * g:4 * g + 4, :])
            # ssq_g[:, 2i:2i+2] = [sum q^2, sum k^2] for tile 4g+i
            ssq_g = stat_pool.tile([P, 8], F32, tag="ssq", name=f"ssq{g}")
            ps_qkvs = []
            for ii in range(4):
                i = 4 * g + ii
                t0 = i * P
                ensure_xt(i - i % 2)
                xt_huge = xt_tiles[i - i % 2]
                xoff = (i % 2) * P
                ps_qkv = ps.tile([P, 3 * HD], F32, tag="ps",
                                 name=f"psqkv{i}")
                for k in range(D // P):
                    nc.tensor.matmul(ps_qkv[:], xt_huge[:, k, xoff:xoff + P],
                                     wqkv_sb[:, k, :],
                                     start=(k == 0), stop=(k == D // P - 1))
                nc.vector.scalar_tensor_tensor(
                    out=v_t[i][:], in0=ps_qkv[:, 2 * HD:3 * HD],
                    scalar=lam_sb[:, 0:1], in1=ve_g[:, ii, :],
                    op0=mybir.AluOpType.mult, op1=mybir.AluOpType.add)
                sq = scr_pool.tile([P, HD], F32, tag="sq")
                nc.scalar.activation(sq[:], ps_qkv[:, 0:HD],
                                     mybir.ActivationFunctionType.Square,
                                     accum_out=ssq_g[:, 2 * ii:2 * ii + 1])
                nc.scalar.activation(sq[:], ps_qkv[:, HD:2 * HD],
                                     mybir.ActivationFunctionType.Square,
                                     accum_out=ssq_g[:, 2 * ii + 1:2 * ii + 2])
                ps_qkvs.append(ps_qkv)
            # prefetch next group's x tiles
            ensure_xt(4 * g + 4)
            ensure_xt(4 * g + 6)
            return ssq_g, ps_qkvs, ve_g, cos_g, sin_g

        def norm_stage(g, ssq_g, ps_qkvs, cos_g, sin_g):
            # rsq = 1/sqrt(ssq) batched for the group: integer magic + 2
            # Newton iterations, all on DVE (no ACT table involvement).
            # 1/sqrt(mean) = rsq * sqrt(HD) is folded into the final scales.
            h_i = stat_pool.tile([P, 8], I32, tag="h_i")
            nc.vector.tensor_scalar(
                out=h_i[:], in0=ssq_g[:].bitcast(I32), scalar1=1,
                scalar2=None,
                op0=mybir.AluOpType.logical_shift_right)
            y0 = stat_pool.tile([P, 8], F32, tag="y0")
            nc.vector.tensor_scalar(
                out=y0[:].bitcast(I32), in0=h_i[:], scalar1=-1,
                scalar2=RSQRT_MAGIC,
                op0=mybir.AluOpType.mult, op1=mybir.AluOpType.add)
            t1 = stat_pool.tile([P, 8], F32, tag="t1")
            rsq = stat_pool.tile([P, 8], F32, tag="rsq", name=f"rsq{g}")
            cur = y0
            for it, nxt in ((0, t1), (1, rsq)):
                tt = stat_pool.tile([P, 8], F32, tag=f"tt{it}")
                nc.vector.tensor_mul(tt[:], cur[:], cur[:])
                nc.vector.tensor_mul(tt[:], tt[:], ssq_g[:])
                nc.vector.tensor_scalar(
                    out=tt[:], in0=tt[:], scalar1=-0.5, scalar2=1.5,
                    op0=mybir.AluOpType.mult, op1=mybir.AluOpType.add)
                nc.vector.tensor_mul(nxt[:], cur[:], tt[:])
                cur = nxt

            sq128 = float(np.sqrt(HD))
            for ii in range(4):
                i = 4 * g + ii
                t0 = i * P
                ps_qkv = ps_qkvs[ii]
                qkn = tin_pool.tile([P, 2 * HD], MMD, tag="qkn",
                                    name=f"qkn{i}")
                nc.vector.tensor_scalar(
                    out=qkn[:, 0:HD], in0=ps_qkv[:, 0:HD],
                    scalar1=rsq[:, 2 * ii:2 * ii + 1],
                    scalar2=ATTN_SCALE * sq128,
                    op0=mybir.AluOpType.mult, op1=mybir.AluOpType.mult)
                nc.vector.tensor_scalar(
                    out=qkn[:, HD:2 * HD], in0=ps_qkv[:, HD:2 * HD],
                    scalar1=rsq[:, 2 * ii + 1:2 * ii + 2], scalar2=sq128,
                    op0=mybir.AluOpType.mult, op1=mybir.AluOpType.mult)
                # rope on first-quarter pairs of q AND k in one op each
                def two_rng(tl, col0):
                    src = tl[:]
                    return bass.AP(src.tensor, src.offset + col0,
                                   [list(src.ap[0]), [HD, 2], [1, QUARTER]])

                def cs_b(ap2d):
                    return bass.AP(ap2d.tensor, ap2d.offset,
                                   [list(ap2d.ap[0]), [0, 2],
                                    list(ap2d.ap[-1])])

                x1 = two_rng(qkn, 0)
                x2 = two_rng(qkn, 2 * QUARTER)
                cb_, sb_ = cs_b(cos_g[:, ii, :]), cs_b(sin_g[:, ii, :])
                a = scr_pool.tile([P, 2, QUARTER], F32, tag="ropeA")
                b = scr_pool.tile([P, 2, QUARTER], F32, tag="ropeB")
                c2 = scr_pool.tile([P, 2, QUARTER], F32, tag="ropeC")
                d2 = scr_pool.tile([P, 2, QUARTER], F32, tag="ropeD")
                nc.vector.tensor_mul(a[:], x1, cb_)
                nc.vector.tensor_mul(b[:], x2, sb_)
                nc.vector.tensor_mul(c2[:], x2, cb_)
                nc.vector.tensor_mul(d2[:], x1, sb_)
                nc.vector.tensor_add(x1, a[:], b[:])
                nc.vector.tensor_sub(x2, c2[:], d2[:])
                # transpose q,k into [e, t] layout (PE transpose, DVE evict)
                sub = ii * P
                for ei, (src_ap, dst, c0) in enumerate(
                        ((qkn[:, 0:HD], qT_c[g], sub),
                         (qkn[:, HD:2 * HD], kT_t[i], 0))):
                    ps_tr = ps.tile([P, P], MMD, tag="ps")
                    nc.tensor.transpose(ps_tr[:], src_ap, ident[:])
                    if ei:
                        nc.vector.tensor_copy(dst[:, c0:c0 + P], ps_tr[:])
                    else:
                        nc.scalar.copy(dst[:, c0:c0 + P], ps_tr[:])

        def attn_chunk(c):
            jmax = 4 * c + 4
            ps_y = psy.tile([P, TCH], F32, tag="psy", name=f"psy{c}")
            acc = acc_pool.tile([P, TCH], MMD, name=f"acc{c}")
            s_psums = {}

            def s_mm(j):
                p_s = ps.tile([P, TCH], F32, tag="ps")
                nc.tensor.matmul(p_s[:], kT_t[j][:], qT_c[c][:],
                                 start=True, stop=True)
                return p_s

            s_psums[0] = s_mm(0)
            for j in range(jmax):
                if j + 1 < jmax:
                    s_psums[j + 1] = s_mm(j + 1)
                if j == 1:
                    flush_den(0)
                p_s = s_psums.pop(j)
                e_sb = exp_pool.tile([P, TCH], MMD)
                nc.scalar.activation(e_sb[:], p_s[:],
                                     mybir.ActivationFunctionType.Exp,
                                     bias=expb_col[:])
                if j >= 4 * c:
                    nc.vector.tensor_mul(e_sb[:], e_sb[:],
                                         masks[j - 4 * c][:])
                if j == 0:
                    nc.vector.tensor_copy(acc[:], e_sb[:])
                else:
                    nc.vector.tensor_add(acc[:], acc[:], e_sb[:])
                nc.tensor.matmul(ps_y[:], v_t[j][:], e_sb[:],
                                 start=(j == 0), stop=(j == jmax - 1))
            ps_r = psy.tile([1, TCH], F32, tag="psy", name=f"psr{c}")
            nc.tensor.matmul(ps_r[:], ones_col[:], acc[:],
                             start=True, stop=True)
            pending_den.append((ps_y, ps_r, c))

        # ---- main loop: qkv(g) | attn(g-1) | norm(g) dovetail ----
        # chunk 0 (4 key-blocks) is processed LAST so the pre-collective
        # tail is as short as possible.
        cpw_tiles = {}
        for g in range(NC_CH):
            ssq_g, ps_qkvs, ve_g, cos_g, sin_g = qkv_stage(g)
            if g >= 2:
                attn_chunk(g - 1)
            norm_stage(g, ssq_g, ps_qkvs, cos_g, sin_g)
            if g == 5:  # prefetch output-projection weights mid-flight
                for dh in range(D // TCH):
                    for h in range(H):
                        ct = cpw_pool.tile([P, TCH], MMD, tag="cpw",
                                           name=f"cpw{h}_{dh}")
                        nc.gpsimd.dma_start(
                            out=ct[:],
                            in_=cpw.ap()[h * P:(h + 1) * P,
                                         dh * TCH:(dh + 1) * TCH])
                        cpw_tiles[(h, dh)] = ct
        attn_chunk(NC_CH - 1)
        attn_chunk(0)
        flush_den(0)

        # ---- AllToAll: head-parallel -> sequence-parallel ----
        nc.gpsimd.collective_compute(
            "AllToAll", mybir.AluOpType.bypass,
            replica_groups=[list(range(N_CORES))],
            ins=[cc_in[:].opt()], outs=[cc_out[:].opt()])
        yall = big.tile([P, N_CORES, TCH], MMD)
        for j in range(N_CORES):
            nc.sync.dma_start(out=yall[:, j, :], in_=cc_out_v[j])

        # ---- output projection for this core's 512 rows ----
        for i in range(SHARD // P):
            for dh in range(D // TCH):
                ps_o = ps.tile([P, TCH], F32, tag="ps")
                for h in range(H):
                    nc.tensor.matmul(ps_o[:], yall[:, h, i * P:(i + 1) * P],
                                     cpw_tiles[(h, dh)][:],
                                     start=(h == 0), stop=(h == H - 1))
                o_sb = exp_pool.tile([P, TCH], F32, tag="osb")
                nc.scalar.copy(o_sb[:], ps_o[:])
                nc.sync.dma_start(
                    out=y_shard.ap()[i * P:(i + 1) * P,
                                     dh * TCH:(dh + 1) * TCH],
                    in_=o_sb[:])

    nc.compile()
    return nc


def _host_prep(x, ve, qkv_w, lambdas, c_proj_w):
    x = np.asarray(x, dtype=np.float32)
    ve = np.asarray(ve, dtype=np.float32)
    qkv_w = np.asarray(qkv_w, dtype=np.float32)
    lambdas = np.asarray(lambdas, dtype=np.float32)
    c_proj_w = np.asarray(c_proj_w, dtype=np.float32)

    xT = np.ascontiguousarray(x[0].T.astype(NP_MMD))
    cpwT = np.ascontiguousarray(c_proj_w.T.astype(NP_MMD))
    lam_b = np.ascontiguousarray(np.broadcast_to(lambdas, (P, 2)))

    angular = (np.float32(1.0 / 1024.0)
               ** np.linspace(0.0, 1.0, QUARTER, dtype=np.float32))
    t = np.arange(T, dtype=np.float32)
    theta = t[:, None] * angular[None, :]
    cos32 = np.cos(theta).astype(np.float32)
    sin32 = np.sin(theta).astype(np.float32)

    in_maps = []
    for h in range(N_CORES):
        sl = slice(h * HD, (h + 1) * HD)
        w_qkvT = np.ascontiguousarray(np.concatenate(
            [qkv_w[0, sl, :].T, qkv_w[1, sl, :].T, qkv_w[2, sl, :].T],
            axis=1).astype(NP_MMD))
        in_maps.append({
            "x_t": xT,
            "w_qkv": w_qkvT,
            "cos_t": cos32,
            "sin_t": sin32,
            "ve_h": np.ascontiguousarray(ve[0][:, sl] * lambdas[1]),
            "lam": lam_b,
            "cpw": cpwT,
        })
    return in_maps


def kernel(x, ve, qkv_w, lambdas, c_proj_w, _trace=False, _trace_kwargs=None):
    if "nc" not in _cached:
        _cached["nc"] = build_module()
    nc = _cached["nc"]
    in_maps = _host_prep(x, ve, qkv_w, lambdas, c_proj_w)
    kw = {}
    if _trace:
        kw = dict(trace=True, **(_trace_kwargs or {}))
    res = run_bass_kernel_spmd(nc, in_maps, core_ids=list(range(N_CORES)),
                               **kw)
    _cached["last_result"] = res
    out = np.concatenate([res.results[c]["y_shard"] for c in range(N_CORES)],
                         axis=0)
    return out[None].astype(np.float32)


# revision 28
# speedup vs baseline: 1.0001x; 1.0001x over previous
"""Trainium2 Bass kernel: causal self-attention (modded-nanogpt style),
tensor-parallel over heads across 8 NeuronCores with an on-device AllToAll
re-shard before the output projection.

Self-contained: hardcodes B=1, T=4096, D=1024, H=8, Hd=128, scale=0.12.

Per-core program (core = head), processed in 8 groups of four 128-row tiles:
  qkv_stage(g)   4x[ qkv matmuls (xT tile stationary), lambda-mix of v,
                     sum-of-squares stats (ACT Square + accum) ]
  attn_chunk(g-1) S^T = kT.T@qT per 128-key-block; ACT exp out of PSUM
                 (2^-12 folded into the bias keeps fp16 in range and cancels
                 in the normalize); denominator = DVE fp16 adds + one
                 ones-matmul; y^T += v.T @ expT
  norm_stage(g)  batched rsqrt via DVE integer magic + 2 Newton steps (no
                 ACT table loads), q/k normalize, RoPE, PE transposes
Then an AllToAll re-shards heads -> sequence and the output projection runs
over all heads for this core's 512 rows.
"""

import os
import sys

sys.path.insert(0, "/opt/trn_rl_repo")

from contextlib import ExitStack

import numpy as np

import concourse.bass as bass
import concourse.bacc as bacc
import concourse.mybir as mybir
import concourse.tile as tile
from concourse.bass_utils import run_bass_kernel_spmd
from concourse.masks import make_identity

N_CORES = 8
T = 4096
D = 1024
H = 8
HD = 128
ATTN_SCALE = 0.12
P = 128
TCH = 512
NT = T // P          # 32 t-tiles
NC_CH = T // TCH     # 8 chunks / tile groups
SHARD = T // N_CORES
QUARTER = HD // 4

F32 = mybir.dt.float32
I32 = mybir.dt.int32
_MODE = os.environ.get("KBASS_MM_DT", "f16")
MMD = {"f32r": mybir.dt.float32r, "f16": mybir.dt.float16,
       "f32": F32}[_MODE]
NP_MMD = {"f32r": np.float32, "f16": np.float16, "f32": np.float32}[_MODE]
# exp(s - 12*ln2) = 2^-12 * exp(s): keeps fp16 exp values and their fp16
# partial sums in range; the scaling cancels in the softmax normalize.
EXP_BIAS = -8.317766166719343 if _MODE == "f16" else 0.0
RSQRT_MAGIC = 0x5F3759DF

_cached = {}


def build_module():
    nc = bacc.Bacc("TRN2", target_bir_lowering=False, debug=False,
                   num_devices=N_CORES)

    x_t = nc.dram_tensor("x_t", [D, T], MMD, kind="ExternalInput")
    w_qkv = nc.dram_tensor("w_qkv", [D, 3 * HD], MMD, kind="ExternalInput")
    cos_t = nc.dram_tensor("cos_t", [T, QUARTER], F32, kind="ExternalInput")
    sin_t = nc.dram_tensor("sin_t", [T, QUARTER], F32, kind="ExternalInput")
    ve_h = nc.dram_tensor("ve_h", [T, HD], F32, kind="ExternalInput")
    lam = nc.dram_tensor("lam", [P, 2], F32, kind="ExternalInput")
    cpw = nc.dram_tensor("cpw", [D, D], MMD, kind="ExternalInput")
    y_shard = nc.dram_tensor("y_shard", [SHARD, D], F32, kind="ExternalOutput")

    with tile.TileContext(nc) as tc, nc.allow_low_precision(
            reason="reduced-precision matmul operands"), ExitStack() as ctx:
        const = ctx.enter_context(tc.tile_pool(name="const", bufs=1))
        wqkv_pool = ctx.enter_context(tc.tile_pool(name="wqkv", bufs=1))
        big = ctx.enter_context(tc.tile_pool(name="big", bufs=1))
        xt_pool = ctx.enter_context(tc.tile_pool(name="xt", bufs=5))
        cs_pool = ctx.enter_context(tc.tile_pool(name="cs", bufs=4))
        ve_pool = ctx.enter_context(tc.tile_pool(name="vein", bufs=3))
        scr_pool = ctx.enter_context(tc.tile_pool(name="scr", bufs=3))
        stat_pool = ctx.enter_context(tc.tile_pool(name="stat", bufs=3))
        tin_pool = ctx.enter_context(tc.tile_pool(name="tin", bufs=10))
        exp_pool = ctx.enter_context(tc.tile_pool(name="exp", bufs=5))
        acc_pool = ctx.enter_context(tc.tile_pool(name="acc", bufs=2))
        rro_pool = ctx.enter_context(tc.tile_pool(name="rro", bufs=2))
        cpw_pool = ctx.enter_context(tc.tile_pool(name="cpw", bufs=16))
        ps = ctx.enter_context(tc.tile_pool(name="ps", bufs=5, space="PSUM"))
        psy = ctx.enter_context(tc.tile_pool(name="psy", bufs=3, space="PSUM"))
        dram = ctx.enter_context(tc.tile_pool(name="dram", bufs=1,
                                              space="DRAM"))

        # ---- weights first so their DMAs lead the queues ----
        wqkv_sb = wqkv_pool.tile([P, D // P, 3 * HD], MMD)
        for k in range(D // P):
            eng = nc.scalar if k % 2 else nc.sync
            eng.dma_start(out=wqkv_sb[:, k, :],
                          in_=w_qkv.ap()[k * P:(k + 1) * P, :])
        lam_sb = const.tile([P, 2], F32)
        nc.scalar.dma_start(out=lam_sb[:], in_=lam.ap())

        # ---- constants ----
        ones_f = const.tile([P, 1], F32)
        nc.vector.memset(ones_f[:], 1.0)
        ones_col = const.tile([P, 1], MMD)
        nc.scalar.copy(ones_col[:], ones_f[:])
        ones_row_f = const.tile([1, P], F32)
        nc.vector.memset(ones_row_f[:], 1.0)
        ones_row = const.tile([1, P], MMD)
        nc.scalar.copy(ones_row[:], ones_row_f[:])
        expb_col = const.tile([P, 1], F32)
        nc.vector.memset(expb_col[:], EXP_BIAS)
        masks = []
        mk_f = const.tile([P, TCH], F32)
        for m in range(4):
            nc.vector.memset(mk_f[:], 1.0)
            nc.gpsimd.affine_select(
                out=mk_f[:], in_=mk_f[:],
                compare_op=mybir.AluOpType.is_ge, fill=0.0,
                base=-128 * m, channel_multiplier=-1, pattern=[[1, TCH]])
            mk = const.tile([P, TCH], MMD, name=f"mask{m}")
            nc.scalar.copy(mk[:], mk_f[:])
            masks.append(mk)
        ident_f = const.tile([P, P], F32)
        make_identity(nc, ident_f)
        ident = const.tile([P, P], MMD)
        nc.scalar.copy(ident[:], ident_f[:])

        # ---- persistent per-block tensors (separate tiles => precise deps)
        kT_t = [big.tile([P, P], MMD, name=f"kT{j}") for j in range(NT)]
        v_t = [big.tile([P, HD], MMD, name=f"v{j}") for j in range(NT)]
        qT_c = [big.tile([P, TCH], MMD, name=f"qT{c}") for c in range(NC_CH)]
        yT_c = [big.tile([P, TCH], MMD, name=f"yT{c}") for c in range(NC_CH)]

        cc_in = dram.tile([N_CORES * P * TCH], MMD)
        cc_out = dram.tile([N_CORES * P * TCH], MMD)
        cc_in_v = cc_in[:].rearrange("(j p f) -> j p f", j=N_CORES, p=P)
        cc_out_v = cc_out[:].rearrange("(j p f) -> j p f", j=N_CORES, p=P)

        xt_tiles = {}

        def ensure_xt(i):  # i even: tile pair (i, i+1)
            if i in xt_tiles or i >= NT:
                return
            xt = xt_pool.tile([P, D // P, 2 * P], MMD, tag="xt",
                              name=f"xt{i}")
            nc.sync.dma_start(
                out=xt[:],
                in_=x_t.ap().rearrange("(k p) t -> p k t", p=P)
                    [:, :, i * P:(i + 2) * P])
            xt_tiles[i] = xt

        pending_den = []

        def flush_den_part(step):
            # steps 0-3: reciprocal quarters (short DVE ops so the DVE queue
            # never stalls long); step 4: broadcast + normalize + ship
            if not pending_den:
                return
            ps_y, ps_r, c, rrow = pending_den[0]
            if step < 4:
                q0 = step * (TCH // 4)
                nc.vector.reciprocal(rrow[0:1, q0:q0 + TCH // 4],
                                     ps_r[0:1, q0:q0 + TCH // 4])
                return
            pending_den.pop(0)
            rb_sb = rro_pool.tile([P, TCH], F32, tag="rb_sb")
            nc.gpsimd.partition_broadcast(rb_sb[:], rrow[:])
            nc.vector.tensor_mul(yT_c[c][:], ps_y[:], rb_sb[:])
            nc.gpsimd.dma_start(out=cc_in_v[c], in_=yT_c[c][:])

        def flush_den(n):
            while len(pending_den) > n:
                for st in range(5):
                    flush_den_part(st)

        def qkv_stage(g):
            ensure_xt(4 * g)
            ensure_xt(4 * g + 2)
            ve_g = ve_pool.tile([P, 4, HD], F32, tag="ve", name=f"ve{g}")
            nc.gpsimd.dma_start(
                out=ve_g[:],
                in_=ve_h.ap().rearrange("(n p) e -> p n e", p=P)
                    [:, 4 * g:4 * g + 4, :])
            cos_g = cs_pool.tile([P, 4, QUARTER], F32, tag="cos",
                                 name=f"cos{g}")
            sin_g = cs_pool.tile([P, 4, QUARTER], F32, tag="sin",
                                 name=f"sin{g}")
            nc.gpsimd.dma_start(
                out=cos_g[:],
                in_=cos_t.ap().rearrange("(n p) e -> p n e", p=P)
                    [:, 4 * g:4 * g + 4, :])
            nc.gpsimd.dma_start(
                out=sin_g[:],
                in_=sin_t.ap().rearrange("(n p) e -> p n e", p=P)
                    [:, 4 * g:4 * g + 4, :])
            # ssq_g[:, 2i:2i+2] = [sum q^2, sum k^2] for tile 4g+i
            ssq_g = stat_pool.tile([P, 8], F32, tag="ssq", name=f"ssq{g}")
            ps_qkvs = []
            for ii in range(4):
                i = 4 * g + ii
                t0 = i * P
                ensure_xt(i - i % 2)
                xt_huge = xt_tiles[i - i % 2]
                xoff = (i % 2) * P
                ps_qkv = ps.tile([P, 3 * HD], F32, tag="ps",
                                 name=f"psqkv{i}")
                for k in range(D // P):
                    nc.tensor.matmul(ps_qkv[:], xt_huge[:, k, xoff:xoff + P],
                                     wqkv_sb[:, k, :],
                                     start=(k == 0), stop=(k == D // P - 1))
                nc.vector.scalar_tensor_tensor(
                    out=v_t[i][:], in0=ps_qkv[:, 2 * HD:3 * HD],
                    scalar=lam_sb[:, 0:1], in1=ve_g[:, ii, :],
                    op0=mybir.AluOpType.mult, op1=mybir.AluOpType.add)
                sq = scr_pool.tile([P, HD], F32, tag="sq")
                nc.scalar.activation(sq[:], ps_qkv[:, 0:HD],
                                     mybir.ActivationFunctionType.Square,
                                     accum_out=ssq_g[:, 2 * ii:2 * ii + 1])
                nc.scalar.activation(sq[:], ps_qkv[:, HD:2 * HD],
                                     mybir.ActivationFunctionType.Square,
                                     accum_out=ssq_g[:, 2 * ii + 1:2 * ii + 2])
                ps_qkvs.append(ps_qkv)
            # prefetch next group's x tiles
            ensure_xt(4 * g + 4)
            ensure_xt(4 * g + 6)
            return ssq_g, ps_qkvs, ve_g, cos_g, sin_g

        def norm_stage(g, ssq_g, ps_qkvs, cos_g, sin_g):
            # rsq = 1/sqrt(ssq) batched for the group: integer magic + 2
            # Newton iterations, all on DVE (no ACT table involvement).
            # 1/sqrt(mean) = rsq * sqrt(HD) is folded into the final scales.
            h_i = stat_pool.tile([P, 8], I32, tag="h_i")
            nc.vector.tensor_scalar(
                out=h_i[:], in0=ssq_g[:].bitcast(I32), scalar1=1,
                scalar2=None,
                op0=mybir.AluOpType.logical_shift_right)
            y0 = stat_pool.tile([P, 8], F32, tag="y0")
            nc.vector.tensor_scalar(
                out=y0[:].bitcast(I32), in0=h_i[:], scalar1=-1,
                scalar2=RSQRT_MAGIC,
                op0=mybir.AluOpType.mult, op1=mybir.AluOpType.add)
            t1 = stat_pool.tile([P, 8], F32, tag="t1")
            rsq = stat_pool.tile([P, 8], F32, tag="rsq", name=f"rsq{g}")
            cur = y0
            for it, nxt in ((0, t1), (1, rsq)):
                tt = stat_pool.tile([P, 8], F32, tag=f"tt{it}")
                nc.vector.tensor_mul(tt[:], cur[:], cur[:])
                nc.vector.tensor_mul(tt[:], tt[:], ssq_g[:])
                nc.vector.tensor_scalar(
                    out=tt[:], in0=tt[:], scalar1=-0.5, scalar2=1.5,
                    op0=mybir.AluOpType.mult, op1=mybir.AluOpType.add)
                nc.vector.tensor_mul(nxt[:], cur[:], tt[:])
                cur = nxt

            sq128 = float(np.sqrt(HD))
            for ii in range(4):
                i = 4 * g + ii
                t0 = i * P
                ps_qkv = ps_qkvs[ii]
                qkn = tin_pool.tile([P, 2 * HD], MMD, tag="qkn",
                                    name=f"qkn{i}")
                nc.vector.tensor_scalar(
                    out=qkn[:, 0:HD], in0=ps_qkv[:, 0:HD],
                    scalar1=rsq[:, 2 * ii:2 * ii + 1],
                    scalar2=ATTN_SCALE * sq128,
                    op0=mybir.AluOpType.mult, op1=mybir.AluOpType.mult)
                nc.vector.tensor_scalar(
                    out=qkn[:, HD:2 * HD], in0=ps_qkv[:, HD:2 * HD],
                    scalar1=rsq[:, 2 * ii + 1:2 * ii + 2], scalar2=sq128,
                    op0=mybir.AluOpType.mult, op1=mybir.AluOpType.mult)
                # rope on first-quarter pairs of q AND k in one op each
                def two_rng(tl, col0):
                    src = tl[:]
                    return bass.AP(src.tensor, src.offset + col0,
                                   [list(src.ap[0]), [HD, 2], [1, QUARTER]])

                def cs_b(ap2d):
                    return bass.AP(ap2d.tensor, ap2d.offset,
                                   [list(ap2d.ap[0]), [0, 2],
                                    list(ap2d.ap[-1])])

                x1 = two_rng(qkn, 0)
                x2 = two_rng(qkn, 2 * QUARTER)
                cb_, sb_ = cs_b(cos_g[:, ii, :]), cs_b(sin_g[:, ii, :])
                a = scr_pool.tile([P, 2, QUARTER], F32, tag="ropeA")
                b = scr_pool.tile([P, 2, QUARTER], F32, tag="ropeB")
                c2 = scr_pool.tile([P, 2, QUARTER], F32, tag="ropeC")
                d2 = scr_pool.tile([P, 2, QUARTER], F32, tag="ropeD")
                nc.vector.tensor_mul(a[:], x1, cb_)
                nc.vector.tensor_mul(b[:], x2, sb_)
                nc.vector.tensor_mul(c2[:], x2, cb_)
                nc.vector.tensor_mul(d2[:], x1, sb_)
                nc.vector.tensor_add(x1, a[:], b[:])
                nc.vector.tensor_sub(x2, c2[:], d2[:])
                # transpose q,k into [e, t] layout (PE transpose, DVE evict)
                sub = ii * P
                for ei, (src_ap, dst, c0) in enumerate(
                        ((qkn[:, 0:HD], qT_c[g], sub),
                         (qkn[:, HD:2 * HD], kT_t[i], 0))):
                    ps_tr = ps.tile([P, P], MMD, tag="ps")
                    nc.tensor.transpose(ps_tr[:], src_ap, ident[:])
                    if ei:
                        nc.vector.tensor_copy(dst[:, c0:c0 + P], ps_tr[:])
                    else:
                        nc.scalar.copy(dst[:, c0:c0 + P], ps_tr[:])

        def attn_chunk(c):
            jmax = 4 * c + 4
            ps_y = psy.tile([P, TCH], F32, tag="psy", name=f"psy{c}")
            acc = acc_pool.tile([P, TCH], MMD, name=f"acc{c}")
            s_psums = {}

            def s_mm(j):
                p_s = ps.tile([P, TCH], F32, tag="ps")
                nc.tensor.matmul(p_s[:], kT_t[j][:], qT_c[c][:],
                                 start=True, stop=True)
                return p_s

            s_psums[0] = s_mm(0)
            for j in range(jmax):
                if j + 1 < jmax:
                    s_psums[j + 1] = s_mm(j + 1)
                if 1 <= j <= 5:
                    flush_den_part(j - 1)
                p_s = s_psums.pop(j)
                e_sb = exp_pool.tile([P, TCH], MMD)
                nc.scalar.activation(e_sb[:], p_s[:],
                                     mybir.ActivationFunctionType.Exp,
                                     bias=expb_col[:])
                if j >= 4 * c:
                    nc.vector.tensor_mul(e_sb[:], e_sb[:],
                                         masks[j - 4 * c][:])
                if j == 0:
                    nc.vector.tensor_copy(acc[:], e_sb[:])
                else:
                    nc.vector.tensor_add(acc[:], acc[:], e_sb[:])
                nc.tensor.matmul(ps_y[:], v_t[j][:], e_sb[:],
                                 start=(j == 0), stop=(j == jmax - 1))
            ps_r = psy.tile([1, TCH], F32, tag="psy", name=f"psr{c}")
            nc.tensor.matmul(ps_r[:], ones_col[:], acc[:],
                             start=True, stop=True)
            rrow = rro_pool.tile([1, TCH], F32, name=f"rrow{c}")
            pending_den.append((ps_y, ps_r, c, rrow))

        # ---- main loop: qkv(g) | attn(g-1) | norm(g) dovetail ----
        # chunk 0 (4 key-blocks) is processed LAST so the pre-collective
        # tail is as short as possible.
        cpw_tiles = {}
        for g in range(NC_CH):
            ssq_g, ps_qkvs, ve_g, cos_g, sin_g = qkv_stage(g)
            if g >= 2:
                attn_chunk(g - 1)
            norm_stage(g, ssq_g, ps_qkvs, cos_g, sin_g)
            if g == 5:  # prefetch output-projection weights mid-flight
                for dh in range(D // TCH):
                    for h in range(H):
                        ct = cpw_pool.tile([P, TCH], MMD, tag="cpw",
                                           name=f"cpw{h}_{dh}")
                        nc.gpsimd.dma_start(
                            out=ct[:],
                            in_=cpw.ap()[h * P:(h + 1) * P,
                                         dh * TCH:(dh + 1) * TCH])
                        cpw_tiles[(h, dh)] = ct
        attn_chunk(NC_CH - 1)
        attn_chunk(0)
        flush_den(0)

        # ---- AllToAll: head-parallel -> sequence-parallel ----
        nc.gpsimd.collective_compute(
            "AllToAll", mybir.AluOpType.bypass,
            replica_groups=[list(range(N_CORES))],
            ins=[cc_in[:].opt()], outs=[cc_out[:].opt()])
        yall = big.tile([P, N_CORES, TCH], MMD)
        for j in range(N_CORES):
            nc.sync.dma_start(out=yall[:, j, :], in_=cc_out_v[j])

        # ---- output projection for this core's 512 rows ----
        for i in range(SHARD // P):
            for dh in range(D // TCH):
                ps_o = ps.tile([P, TCH], F32, tag="ps")
                for h in range(H):
                    nc.tensor.matmul(ps_o[:], yall[:, h, i * P:(i + 1) * P],
                                     cpw_tiles[(h, dh)][:],
                                     start=(h == 0), stop=(h == H - 1))
                o_sb = exp_pool.tile([P, TCH], F32, tag="osb")
                nc.scalar.copy(o_sb[:], ps_o[:])
                nc.sync.dma_start(
                    out=y_shard.ap()[i * P:(i + 1) * P,
                                     dh * TCH:(dh + 1) * TCH],
                    in_=o_sb[:])

    nc.compile()
    return nc


def _host_prep(x, ve, qkv_w, lambdas, c_proj_w):
    x = np.asarray(x, dtype=np.float32)
    ve = np.asarray(ve, dtype=np.float32)
    qkv_w = np.asarray(qkv_w, dtype=np.float32)
    lambdas = np.asarray(lambdas, dtype=np.float32)
    c_proj_w = np.asarray(c_proj_w, dtype=np.float32)

    xT = np.ascontiguousarray(x[0].T.astype(NP_MMD))
    cpwT = np.ascontiguousarray(c_proj_w.T.astype(NP_MMD))
    lam_b = np.ascontiguousarray(np.broadcast_to(lambdas, (P, 2)))

    angular = (np.float32(1.0 / 1024.0)
               ** np.linspace(0.0, 1.0, QUARTER, dtype=np.float32))
    t = np.arange(T, dtype=np.float32)
    theta = t[:, None] * angular[None, :]
    cos32 = np.cos(theta).astype(np.float32)
    sin32 = np.sin(theta).astype(np.float32)

    in_maps = []
    for h in range(N_CORES):
        sl = slice(h * HD, (h + 1) * HD)
        w_qkvT = np.ascontiguousarray(np.concatenate(
            [qkv_w[0, sl, :].T, qkv_w[1, sl, :].T, qkv_w[2, sl, :].T],
            axis=1).astype(NP_MMD))
        in_maps.append({
            "x_t": xT,
            "w_qkv": w_qkvT,
            "cos_t": cos32,
            "sin_t": sin32,
            "ve_h": np.ascontiguousarray(ve[0][:, sl] * lambdas[1]),
            "lam": lam_b,
            "cpw": cpwT,
        })
    return in_maps


def kernel(x, ve, qkv_w, lambdas, c_proj_w, _trace=False, _trace_kwargs=None):
    if "nc" not in _cached:
        _cached["nc"] = build_module()
    nc = _cached["nc"]
    in_maps = _host_prep(x, ve, qkv_w, lambdas, c_proj_w)
    kw = {}
    if _trace:
        kw = dict(trace=True, **(_trace_kwargs or {}))
    res = run_bass_kernel_spmd(nc, in_maps, core_ids=list(range(N_CORES)),
                               **kw)
    _cached["last_result"] = res
    out = np.concatenate([res.results[c]["y_shard"] for c in range(N_CORES)],
                         axis=0)
    return out[None].astype(np.float32)


# revision 29
# speedup vs baseline: 1.1534x; 1.1534x over previous
"""Trainium2 Bass kernel: causal self-attention (modded-nanogpt style),
tensor-parallel over heads across 8 NeuronCores with an on-device AllToAll
re-shard before the output projection.

Self-contained: hardcodes B=1, T=4096, D=1024, H=8, Hd=128, scale=0.12.

Per-core program (core = head), processed in 8 groups of four 128-row tiles:
  qkv_stage(g)   4x[ qkv matmuls (xT tile stationary), lambda-mix of v,
                     sum-of-squares stats (ACT Square + accum) ]
  attn_chunk(g-1) S^T = kT.T@qT per 128-key-block; ACT exp out of PSUM
                 (2^-12 folded into the bias keeps fp16 in range and cancels
                 in the normalize); denominator = DVE fp16 adds + one
                 ones-matmul; y^T += v.T @ expT
  norm_stage(g)  batched rsqrt via DVE integer magic + 2 Newton steps (no
                 ACT table loads), q/k normalize, RoPE, PE transposes
Then an AllToAll re-shards heads -> sequence and the output projection runs
over all heads for this core's 512 rows.
"""

import os
import sys

sys.path.insert(0, "/opt/trn_rl_repo")

from contextlib import ExitStack

import numpy as np

import concourse.bass as bass
import concourse.bacc as bacc
import concourse.mybir as mybir
import concourse.tile as tile
from concourse.bass_utils import run_bass_kernel_spmd
from concourse.masks import make_identity

N_CORES = 8
T = 4096
D = 1024
H = 8
HD = 128
ATTN_SCALE = 0.12
P = 128
TCH = 512
NT = T // P          # 32 t-tiles
NC_CH = T // TCH     # 8 chunks / tile groups
SHARD = T // N_CORES
QUARTER = HD // 4

F32 = mybir.dt.float32
I32 = mybir.dt.int32
_MODE = os.environ.get("KBASS_MM_DT", "f16")
MMD = {"f32r": mybir.dt.float32r, "f16": mybir.dt.float16,
       "f32": F32}[_MODE]
NP_MMD = {"f32r": np.float32, "f16": np.float16, "f32": np.float32}[_MODE]
# exp(s - 12*ln2) = 2^-12 * exp(s): keeps fp16 exp values and their fp16
# partial sums in range; the scaling cancels in the softmax normalize.
EXP_BIAS = -8.317766166719343 if _MODE == "f16" else 0.0
RSQRT_MAGIC = 0x5F3759DF

_cached = {}


def build_module():
    nc = bacc.Bacc("TRN2", target_bir_lowering=False, debug=False,
                   num_devices=N_CORES)

    x_t = nc.dram_tensor("x_t", [D, T], MMD, kind="ExternalInput")
    w_qkv = nc.dram_tensor("w_qkv", [D, 3 * HD], MMD, kind="ExternalInput")
    cos_t = nc.dram_tensor("cos_t", [T, QUARTER], F32, kind="ExternalInput")
    sin_t = nc.dram_tensor("sin_t", [T, QUARTER], F32, kind="ExternalInput")
    ve_h = nc.dram_tensor("ve_h", [T, HD], F32, kind="ExternalInput")
    lam = nc.dram_tensor("lam", [P, 2], F32, kind="ExternalInput")
    cpw = nc.dram_tensor("cpw", [D, D], MMD, kind="ExternalInput")
    y_shard = nc.dram_tensor("y_shard", [SHARD, D], F32, kind="ExternalOutput")

    with tile.TileContext(nc) as tc, nc.allow_low_precision(
            reason="reduced-precision matmul operands"), ExitStack() as ctx:
        const = ctx.enter_context(tc.tile_pool(name="const", bufs=1))
        wqkv_pool = ctx.enter_context(tc.tile_pool(name="wqkv", bufs=1))
        big = ctx.enter_context(tc.tile_pool(name="big", bufs=1))
        xt_pool = ctx.enter_context(tc.tile_pool(name="xt", bufs=7))
        cs_pool = ctx.enter_context(tc.tile_pool(name="cs", bufs=6))
        ve_pool = ctx.enter_context(tc.tile_pool(name="vein", bufs=3))
        scr_pool = ctx.enter_context(tc.tile_pool(name="scr", bufs=3))
        stat_pool = ctx.enter_context(tc.tile_pool(name="stat", bufs=4))
        qk_pool = ctx.enter_context(tc.tile_pool(name="qksb", bufs=12))
        tin_pool = ctx.enter_context(tc.tile_pool(name="tin", bufs=10))
        exp_pool = ctx.enter_context(tc.tile_pool(name="exp", bufs=5))
        acc_pool = ctx.enter_context(tc.tile_pool(name="acc", bufs=2))
        rro_pool = ctx.enter_context(tc.tile_pool(name="rro", bufs=2))
        cpw_pool = ctx.enter_context(tc.tile_pool(name="cpw", bufs=16))
        ps = ctx.enter_context(tc.tile_pool(name="ps", bufs=5, space="PSUM"))
        psy = ctx.enter_context(tc.tile_pool(name="psy", bufs=3, space="PSUM"))
        dram = ctx.enter_context(tc.tile_pool(name="dram", bufs=1,
                                              space="DRAM"))

        # ---- weights first so their DMAs lead the queues ----
        wqkv_sb = wqkv_pool.tile([P, D // P, 3 * HD], MMD)
        for k in range(D // P):
            eng = nc.scalar if k % 2 else nc.sync
            eng.dma_start(out=wqkv_sb[:, k, :],
                          in_=w_qkv.ap()[k * P:(k + 1) * P, :])
        lam_sb = const.tile([P, 2], F32)
        nc.scalar.dma_start(out=lam_sb[:], in_=lam.ap())

        # ---- constants ----
        ones_f = const.tile([P, 1], F32)
        nc.vector.memset(ones_f[:], 1.0)
        ones_col = const.tile([P, 1], MMD)
        nc.scalar.copy(ones_col[:], ones_f[:])
        ones_row_f = const.tile([1, P], F32)
        nc.vector.memset(ones_row_f[:], 1.0)
        ones_row = const.tile([1, P], MMD)
        nc.scalar.copy(ones_row[:], ones_row_f[:])
        expb_col = const.tile([P, 1], F32)
        nc.vector.memset(expb_col[:], EXP_BIAS)
        masks = []
        mk_f = const.tile([P, TCH], F32)
        for m in range(4):
            nc.vector.memset(mk_f[:], 1.0)
            nc.gpsimd.affine_select(
                out=mk_f[:], in_=mk_f[:],
                compare_op=mybir.AluOpType.is_ge, fill=0.0,
                base=-128 * m, channel_multiplier=-1, pattern=[[1, TCH]])
            mk = const.tile([P, TCH], MMD, name=f"mask{m}")
            nc.scalar.copy(mk[:], mk_f[:])
            masks.append(mk)
        ident_f = const.tile([P, P], F32)
        make_identity(nc, ident_f)
        ident = const.tile([P, P], MMD)
        nc.scalar.copy(ident[:], ident_f[:])

        # ---- persistent per-block tensors (separate tiles => precise deps)
        kT_t = [big.tile([P, P], MMD, name=f"kT{j}") for j in range(NT)]
        v_t = [big.tile([P, HD], MMD, name=f"v{j}") for j in range(NT)]
        qT_c = [big.tile([P, TCH], MMD, name=f"qT{c}") for c in range(NC_CH)]
        yT_c = [big.tile([P, TCH], MMD, name=f"yT{c}") for c in range(NC_CH)]

        cc_in = dram.tile([N_CORES * P * TCH], MMD)
        cc_out = dram.tile([N_CORES * P * TCH], MMD)
        cc_in_v = cc_in[:].rearrange("(j p f) -> j p f", j=N_CORES, p=P)
        cc_out_v = cc_out[:].rearrange("(j p f) -> j p f", j=N_CORES, p=P)

        xt_tiles = {}

        def ensure_xt(i):  # i even: tile pair (i, i+1)
            if i in xt_tiles or i >= NT:
                return
            xt = xt_pool.tile([P, D // P, 2 * P], MMD, tag="xt",
                              name=f"xt{i}")
            nc.sync.dma_start(
                out=xt[:],
                in_=x_t.ap().rearrange("(k p) t -> p k t", p=P)
                    [:, :, i * P:(i + 2) * P])
            xt_tiles[i] = xt

        pending_den = []

        def flush_den_part(step):
            # steps 0-3: reciprocal quarters (short DVE ops so the DVE queue
            # never stalls long); step 4: broadcast + normalize + ship
            if not pending_den:
                return
            ps_y, ps_r, c, rrow = pending_den[0]
            if step < 4:
                q0 = step * (TCH // 4)
                nc.vector.reciprocal(rrow[0:1, q0:q0 + TCH // 4],
                                     ps_r[0:1, q0:q0 + TCH // 4])
                return
            pending_den.pop(0)
            rb_sb = rro_pool.tile([P, TCH], F32, tag="rb_sb")
            nc.gpsimd.partition_broadcast(rb_sb[:], rrow[:])
            nc.vector.tensor_mul(yT_c[c][:], ps_y[:], rb_sb[:])
            nc.gpsimd.dma_start(out=cc_in_v[c], in_=yT_c[c][:])

        def flush_den(n):
            while len(pending_den) > n:
                for st in range(5):
                    flush_den_part(st)

        def qkv_stage(g):
            ensure_xt(4 * g)
            ensure_xt(4 * g + 2)
            ve_g = ve_pool.tile([P, 4, HD], F32, tag="ve", name=f"ve{g}")
            nc.gpsimd.dma_start(
                out=ve_g[:],
                in_=ve_h.ap().rearrange("(n p) e -> p n e", p=P)
                    [:, 4 * g:4 * g + 4, :])
            cos_g = cs_pool.tile([P, 4, QUARTER], F32, tag="cos",
                                 name=f"cos{g}")
            sin_g = cs_pool.tile([P, 4, QUARTER], F32, tag="sin",
                                 name=f"sin{g}")
            nc.gpsimd.dma_start(
                out=cos_g[:],
                in_=cos_t.ap().rearrange("(n p) e -> p n e", p=P)
                    [:, 4 * g:4 * g + 4, :])
            nc.gpsimd.dma_start(
                out=sin_g[:],
                in_=sin_t.ap().rearrange("(n p) e -> p n e", p=P)
                    [:, 4 * g:4 * g + 4, :])
            # ssq_g[:, 2i:2i+2] = [sum q^2, sum k^2] for tile 4g+i
            ssq_g = stat_pool.tile([P, 8], F32, tag="ssq", name=f"ssq{g}")
            ps_qkvs = []
            for ii in range(4):
                i = 4 * g + ii
                t0 = i * P
                ensure_xt(i - i % 2)
                xt_huge = xt_tiles[i - i % 2]
                xoff = (i % 2) * P
                ps_qkv = ps.tile([P, 3 * HD], F32, tag="ps",
                                 name=f"psqkv{i}")
                for k in range(D // P):
                    nc.tensor.matmul(ps_qkv[:], xt_huge[:, k, xoff:xoff + P],
                                     wqkv_sb[:, k, :],
                                     start=(k == 0), stop=(k == D // P - 1))
                nc.vector.scalar_tensor_tensor(
                    out=v_t[i][:], in0=ps_qkv[:, 2 * HD:3 * HD],
                    scalar=lam_sb[:, 0:1], in1=ve_g[:, ii, :],
                    op0=mybir.AluOpType.mult, op1=mybir.AluOpType.add)
                qk_sb = qk_pool.tile([P, 2 * HD], F32, tag="qksb",
                                     name=f"qksb{i}")
                if ii % 2:
                    nc.vector.tensor_copy(qk_sb[:], ps_qkv[:, 0:2 * HD])
                else:
                    nc.scalar.copy(qk_sb[:], ps_qkv[:, 0:2 * HD])
                sq = scr_pool.tile([P, HD], F32, tag="sq")
                nc.scalar.activation(sq[:], qk_sb[:, 0:HD],
                                     mybir.ActivationFunctionType.Square,
                                     accum_out=ssq_g[:, 2 * ii:2 * ii + 1])
                nc.scalar.activation(sq[:], qk_sb[:, HD:2 * HD],
                                     mybir.ActivationFunctionType.Square,
                                     accum_out=ssq_g[:, 2 * ii + 1:2 * ii + 2])
                ps_qkvs.append(qk_sb)
            # prefetch next group's x tiles
            ensure_xt(4 * g + 4)
            ensure_xt(4 * g + 6)
            return ssq_g, ps_qkvs, ve_g, cos_g, sin_g

        def norm_stage(g, ssq_g, ps_qkvs, cos_g, sin_g):
            # rsq = 1/sqrt(ssq) batched for the group: integer magic + 2
            # Newton iterations, all on DVE (no ACT table involvement).
            # 1/sqrt(mean) = rsq * sqrt(HD) is folded into the final scales.
            h_i = stat_pool.tile([P, 8], I32, tag="h_i")
            nc.vector.tensor_scalar(
                out=h_i[:], in0=ssq_g[:].bitcast(I32), scalar1=1,
                scalar2=None,
                op0=mybir.AluOpType.logical_shift_right)
            y0 = stat_pool.tile([P, 8], F32, tag="y0")
            nc.vector.tensor_scalar(
                out=y0[:].bitcast(I32), in0=h_i[:], scalar1=-1,
                scalar2=RSQRT_MAGIC,
                op0=mybir.AluOpType.mult, op1=mybir.AluOpType.add)
            t1 = stat_pool.tile([P, 8], F32, tag="t1")
            rsq = stat_pool.tile([P, 8], F32, tag="rsq", name=f"rsq{g}")
            cur = y0
            for it, nxt in ((0, t1), (1, rsq)):
                tt = stat_pool.tile([P, 8], F32, tag=f"tt{it}")
                nc.vector.tensor_mul(tt[:], cur[:], cur[:])
                nc.vector.tensor_mul(tt[:], tt[:], ssq_g[:])
                nc.vector.tensor_scalar(
                    out=tt[:], in0=tt[:], scalar1=-0.5, scalar2=1.5,
                    op0=mybir.AluOpType.mult, op1=mybir.AluOpType.add)
                nc.vector.tensor_mul(nxt[:], cur[:], tt[:])
                cur = nxt

            sq128 = float(np.sqrt(HD))
            for ii in range(4):
                i = 4 * g + ii
                t0 = i * P
                qk_sb = ps_qkvs[ii]
                qkn = tin_pool.tile([P, 2 * HD], MMD, tag="qkn",
                                    name=f"qkn{i}")
                nc.vector.tensor_scalar(
                    out=qkn[:, 0:HD], in0=qk_sb[:, 0:HD],
                    scalar1=rsq[:, 2 * ii:2 * ii + 1],
                    scalar2=ATTN_SCALE * sq128,
                    op0=mybir.AluOpType.mult, op1=mybir.AluOpType.mult)
                nc.vector.tensor_scalar(
                    out=qkn[:, HD:2 * HD], in0=qk_sb[:, HD:2 * HD],
                    scalar1=rsq[:, 2 * ii + 1:2 * ii + 2], scalar2=sq128,
                    op0=mybir.AluOpType.mult, op1=mybir.AluOpType.mult)
                # rope on first-quarter pairs of q AND k in one op each
                def two_rng(tl, col0):
                    src = tl[:]
                    return bass.AP(src.tensor, src.offset + col0,
                                   [list(src.ap[0]), [HD, 2], [1, QUARTER]])

                def cs_b(ap2d):
                    return bass.AP(ap2d.tensor, ap2d.offset,
                                   [list(ap2d.ap[0]), [0, 2],
                                    list(ap2d.ap[-1])])

                x1 = two_rng(qkn, 0)
                x2 = two_rng(qkn, 2 * QUARTER)
                cb_, sb_ = cs_b(cos_g[:, ii, :]), cs_b(sin_g[:, ii, :])
                a = scr_pool.tile([P, 2, QUARTER], F32, tag="ropeA")
                b = scr_pool.tile([P, 2, QUARTER], F32, tag="ropeB")
                c2 = scr_pool.tile([P, 2, QUARTER], F32, tag="ropeC")
                d2 = scr_pool.tile([P, 2, QUARTER], F32, tag="ropeD")
                nc.vector.tensor_mul(a[:], x1, cb_)
                nc.vector.tensor_mul(b[:], x2, sb_)
                nc.vector.tensor_mul(c2[:], x2, cb_)
                nc.vector.tensor_mul(d2[:], x1, sb_)
                nc.vector.tensor_add(x1, a[:], b[:])
                nc.vector.tensor_sub(x2, c2[:], d2[:])
                # transpose q,k into [e, t] layout (PE transpose, DVE evict)
                sub = ii * P
                for ei, (src_ap, dst, c0) in enumerate(
                        ((qkn[:, 0:HD], qT_c[g], sub),
                         (qkn[:, HD:2 * HD], kT_t[i], 0))):
                    ps_tr = ps.tile([P, P], MMD, tag="ps")
                    nc.tensor.transpose(ps_tr[:], src_ap, ident[:])
                    if ei:
                        nc.vector.tensor_copy(dst[:, c0:c0 + P], ps_tr[:])
                    else:
                        nc.scalar.copy(dst[:, c0:c0 + P], ps_tr[:])

        def attn_chunk(c):
            jmax = 4 * c + 4
            ps_y = psy.tile([P, TCH], F32, tag="psy", name=f"psy{c}")
            acc = acc_pool.tile([P, TCH], MMD, name=f"acc{c}")
            s_psums = {}

            def s_mm(j):
                p_s = ps.tile([P, TCH], F32, tag="ps")
                nc.tensor.matmul(p_s[:], kT_t[j][:], qT_c[c][:],
                                 start=True, stop=True)
                return p_s

            s_psums[0] = s_mm(0)
            for j in range(jmax):
                if j + 1 < jmax:
                    s_psums[j + 1] = s_mm(j + 1)
                if 1 <= j <= 5:
                    flush_den_part(j - 1)
                p_s = s_psums.pop(j)
                e_sb = exp_pool.tile([P, TCH], MMD)
                nc.scalar.activation(e_sb[:], p_s[:],
                                     mybir.ActivationFunctionType.Exp,
                                     bias=expb_col[:])
                if j >= 4 * c:
                    nc.vector.tensor_mul(e_sb[:], e_sb[:],
                                         masks[j - 4 * c][:])
                if j == 0:
                    nc.vector.tensor_copy(acc[:], e_sb[:])
                else:
                    nc.vector.tensor_add(acc[:], acc[:], e_sb[:])
                nc.tensor.matmul(ps_y[:], v_t[j][:], e_sb[:],
                                 start=(j == 0), stop=(j == jmax - 1))
            ps_r = psy.tile([1, TCH], F32, tag="psy", name=f"psr{c}")
            nc.tensor.matmul(ps_r[:], ones_col[:], acc[:],
                             start=True, stop=True)
            rrow = rro_pool.tile([1, TCH], F32, name=f"rrow{c}")
            pending_den.append((ps_y, ps_r, c, rrow))

        # ---- main loop: qkv(g) | attn(g-1) | norm(g) dovetail ----
        # chunk 0 (4 key-blocks) is processed LAST so the pre-collective
        # tail is as short as possible.
        cpw_tiles = {}
        handles = {0: qkv_stage(0), 1: qkv_stage(1)}
        for g in range(NC_CH):
            if g >= 2:
                attn_chunk(g - 1)
            if g + 2 < NC_CH:
                handles[g + 2] = qkv_stage(g + 2)
            ssq_g, ps_qkvs, ve_g, cos_g, sin_g = handles.pop(g)
            norm_stage(g, ssq_g, ps_qkvs, cos_g, sin_g)
            if g == 5:  # prefetch output-projection weights mid-flight
                for dh in range(D // TCH):
                    for h in range(H):
                        ct = cpw_pool.tile([P, TCH], MMD, tag="cpw",
                                           name=f"cpw{h}_{dh}")
                        nc.gpsimd.dma_start(
                            out=ct[:],
                            in_=cpw.ap()[h * P:(h + 1) * P,
                                         dh * TCH:(dh + 1) * TCH])
                        cpw_tiles[(h, dh)] = ct
        attn_chunk(NC_CH - 1)
        attn_chunk(0)
        flush_den(0)

        # ---- AllToAll: head-parallel -> sequence-parallel ----
        nc.gpsimd.collective_compute(
            "AllToAll", mybir.AluOpType.bypass,
            replica_groups=[list(range(N_CORES))],
            ins=[cc_in[:].opt()], outs=[cc_out[:].opt()])
        yall = big.tile([P, N_CORES, TCH], MMD)
        for j in range(N_CORES):
            nc.sync.dma_start(out=yall[:, j, :], in_=cc_out_v[j])

        # ---- output projection for this core's 512 rows ----
        for i in range(SHARD // P):
            for dh in range(D // TCH):
                ps_o = ps.tile([P, TCH], F32, tag="ps")
                for h in range(H):
                    nc.tensor.matmul(ps_o[:], yall[:, h, i * P:(i + 1) * P],
                                     cpw_tiles[(h, dh)][:],
                                     start=(h == 0), stop=(h == H - 1))
                o_sb = exp_pool.tile([P, TCH], F32, tag="osb")
                nc.scalar.copy(o_sb[:], ps_o[:])
                nc.sync.dma_start(
                    out=y_shard.ap()[i * P:(i + 1) * P,
                                     dh * TCH:(dh + 1) * TCH],
                    in_=o_sb[:])

    nc.compile()
    return nc


def _host_prep(x, ve, qkv_w, lambdas, c_proj_w):
    x = np.asarray(x, dtype=np.float32)
    ve = np.asarray(ve, dtype=np.float32)
    qkv_w = np.asarray(qkv_w, dtype=np.float32)
    lambdas = np.asarray(lambdas, dtype=np.float32)
    c_proj_w = np.asarray(c_proj_w, dtype=np.float32)

    xT = np.ascontiguousarray(x[0].T.astype(NP_MMD))
    cpwT = np.ascontiguousarray(c_proj_w.T.astype(NP_MMD))
    lam_b = np.ascontiguousarray(np.broadcast_to(lambdas, (P, 2)))

    angular = (np.float32(1.0 / 1024.0)
               ** np.linspace(0.0, 1.0, QUARTER, dtype=np.float32))
    t = np.arange(T, dtype=np.float32)
    theta = t[:, None] * angular[None, :]
    cos32 = np.cos(theta).astype(np.float32)
    sin32 = np.sin(theta).astype(np.float32)

    in_maps = []
    for h in range(N_CORES):
        sl = slice(h * HD, (h + 1) * HD)
        w_qkvT = np.ascontiguousarray(np.concatenate(
            [qkv_w[0, sl, :].T, qkv_w[1, sl, :].T, qkv_w[2, sl, :].T],
            axis=1).astype(NP_MMD))
        in_maps.append({
            "x_t": xT,
            "w_qkv": w_qkvT,
            "cos_t": cos32,
            "sin_t": sin32,
            "ve_h": np.ascontiguousarray(ve[0][:, sl] * lambdas[1]),
            "lam": lam_b,
            "cpw": cpwT,
        })
    return in_maps


def kernel(x, ve, qkv_w, lambdas, c_proj_w, _trace=False, _trace_kwargs=None):
    if "nc" not in _cached:
        _cached["nc"] = build_module()
    nc = _cached["nc"]
    in_maps = _host_prep(x, ve, qkv_w, lambdas, c_proj_w)
    kw = {}
    if _trace:
        kw = dict(trace=True, **(_trace_kwargs or {}))
    res = run_bass_kernel_spmd(nc, in_maps, core_ids=list(range(N_CORES)),
                               **kw)
    _cached["last_result"] = res
    out = np.concatenate([res.results[c]["y_shard"] for c in range(N_CORES)],
                         axis=0)
    return out[None].astype(np.float32)


# revision 30
# speedup vs baseline: 1.1807x; 1.0237x over previous
"""Trainium2 Bass kernel: causal self-attention (modded-nanogpt style),
tensor-parallel over heads across 8 NeuronCores with an on-device AllToAll
re-shard before the output projection.

Self-contained: hardcodes B=1, T=4096, D=1024, H=8, Hd=128, scale=0.12.

Per-core program (core = head), processed in 8 groups of four 128-row tiles:
  qkv_stage(g)   4x[ qkv matmuls (xT tile stationary), lambda-mix of v,
                     sum-of-squares stats (ACT Square + accum) ]
  attn_chunk(g-1) S^T = kT.T@qT per 128-key-block; ACT exp out of PSUM
                 (2^-12 folded into the bias keeps fp16 in range and cancels
                 in the normalize); denominator = DVE fp16 adds + one
                 ones-matmul; y^T += v.T @ expT
  norm_stage(g)  batched rsqrt via DVE integer magic + 2 Newton steps (no
                 ACT table loads), q/k normalize, RoPE, PE transposes
Then an AllToAll re-shards heads -> sequence and the output projection runs
over all heads for this core's 512 rows.
"""

import os
import sys

sys.path.insert(0, "/opt/trn_rl_repo")

from contextlib import ExitStack

import numpy as np

import concourse.bass as bass
import concourse.bacc as bacc
import concourse.mybir as mybir
import concourse.tile as tile
from concourse.bass_utils import run_bass_kernel_spmd
from concourse.masks import make_identity

N_CORES = 8
T = 4096
D = 1024
H = 8
HD = 128
ATTN_SCALE = 0.12
P = 128
TCH = 512
NT = T // P          # 32 t-tiles
NC_CH = T // TCH     # 8 chunks / tile groups
SHARD = T // N_CORES
QUARTER = HD // 4

F32 = mybir.dt.float32
I32 = mybir.dt.int32
_MODE = os.environ.get("KBASS_MM_DT", "f16")
MMD = {"f32r": mybir.dt.float32r, "f16": mybir.dt.float16,
       "f32": F32}[_MODE]
NP_MMD = {"f32r": np.float32, "f16": np.float16, "f32": np.float32}[_MODE]
# exp(s - 12*ln2) = 2^-12 * exp(s): keeps fp16 exp values and their fp16
# partial sums in range; the scaling cancels in the softmax normalize.
EXP_BIAS = -8.317766166719343 if _MODE == "f16" else 0.0
RSQRT_MAGIC = 0x5F3759DF

_cached = {}


def build_module():
    nc = bacc.Bacc("TRN2", target_bir_lowering=False, debug=False,
                   num_devices=N_CORES)

    x_t = nc.dram_tensor("x_t", [D, T], MMD, kind="ExternalInput")
    w_qkv = nc.dram_tensor("w_qkv", [D, 3 * HD], MMD, kind="ExternalInput")
    cos_t = nc.dram_tensor("cos_t", [T, QUARTER], MMD, kind="ExternalInput")
    sin_t = nc.dram_tensor("sin_t", [T, QUARTER], MMD, kind="ExternalInput")
    ve_h = nc.dram_tensor("ve_h", [T, HD], F32, kind="ExternalInput")
    lam = nc.dram_tensor("lam", [P, 2], F32, kind="ExternalInput")
    cpw = nc.dram_tensor("cpw", [D, D], MMD, kind="ExternalInput")
    y_shard = nc.dram_tensor("y_shard", [SHARD, D], F32, kind="ExternalOutput")

    with tile.TileContext(nc) as tc, nc.allow_low_precision(
            reason="reduced-precision matmul operands"), ExitStack() as ctx:
        const = ctx.enter_context(tc.tile_pool(name="const", bufs=1))
        wqkv_pool = ctx.enter_context(tc.tile_pool(name="wqkv", bufs=1))
        big = ctx.enter_context(tc.tile_pool(name="big", bufs=1))
        xt_pool = ctx.enter_context(tc.tile_pool(name="xt", bufs=7))
        cs_pool = ctx.enter_context(tc.tile_pool(name="cs", bufs=6))
        ve_pool = ctx.enter_context(tc.tile_pool(name="vein", bufs=3))
        scr_pool = ctx.enter_context(tc.tile_pool(name="scr", bufs=3))
        stat_pool = ctx.enter_context(tc.tile_pool(name="stat", bufs=4))
        qk_pool = ctx.enter_context(tc.tile_pool(name="qksb", bufs=12))
        tin_pool = ctx.enter_context(tc.tile_pool(name="tin", bufs=10))
        exp_pool = ctx.enter_context(tc.tile_pool(name="exp", bufs=5))
        acc_pool = ctx.enter_context(tc.tile_pool(name="acc", bufs=2))
        rro_pool = ctx.enter_context(tc.tile_pool(name="rro", bufs=2))
        cpw_pool = ctx.enter_context(tc.tile_pool(name="cpw", bufs=16))
        ps = ctx.enter_context(tc.tile_pool(name="ps", bufs=5, space="PSUM"))
        psy = ctx.enter_context(tc.tile_pool(name="psy", bufs=3, space="PSUM"))
        dram = ctx.enter_context(tc.tile_pool(name="dram", bufs=1,
                                              space="DRAM"))

        # ---- weights first so their DMAs lead the queues ----
        wqkv_sb = wqkv_pool.tile([P, D // P, 3 * HD], MMD)
        for k in range(D // P):
            nc.scalar.dma_start(out=wqkv_sb[:, k, :],
                                in_=w_qkv.ap()[k * P:(k + 1) * P, :])
        lam_sb = const.tile([P, 2], F32)
        nc.scalar.dma_start(out=lam_sb[:], in_=lam.ap())

        # ---- constants ----
        ones_f = const.tile([P, 1], F32)
        nc.vector.memset(ones_f[:], 1.0)
        ones_col = const.tile([P, 1], MMD)
        nc.scalar.copy(ones_col[:], ones_f[:])
        ones_row_f = const.tile([1, P], F32)
        nc.vector.memset(ones_row_f[:], 1.0)
        ones_row = const.tile([1, P], MMD)
        nc.scalar.copy(ones_row[:], ones_row_f[:])
        expb_col = const.tile([P, 1], F32)
        nc.vector.memset(expb_col[:], EXP_BIAS)
        masks = []
        mk_f = const.tile([P, TCH], F32)
        for m in range(4):
            nc.vector.memset(mk_f[:], 1.0)
            nc.gpsimd.affine_select(
                out=mk_f[:], in_=mk_f[:],
                compare_op=mybir.AluOpType.is_ge, fill=0.0,
                base=-128 * m, channel_multiplier=-1, pattern=[[1, TCH]])
            mk = const.tile([P, TCH], MMD, name=f"mask{m}")
            nc.scalar.copy(mk[:], mk_f[:])
            masks.append(mk)
        ident_f = const.tile([P, P], F32)
        make_identity(nc, ident_f)
        ident = const.tile([P, P], MMD)
        nc.scalar.copy(ident[:], ident_f[:])

        # ---- persistent per-block tensors (separate tiles => precise deps)
        kT_t = [big.tile([P, P], MMD, name=f"kT{j}") for j in range(NT)]
        v_t = [big.tile([P, HD], MMD, name=f"v{j}") for j in range(NT)]
        qT_c = [big.tile([P, TCH], MMD, name=f"qT{c}") for c in range(NC_CH)]
        yT_c = [big.tile([P, TCH], MMD, name=f"yT{c}") for c in range(NC_CH)]

        cc_in = dram.tile([N_CORES * P * TCH], MMD)
        cc_out = dram.tile([N_CORES * P * TCH], MMD)
        cc_in_v = cc_in[:].rearrange("(j p f) -> j p f", j=N_CORES, p=P)
        cc_out_v = cc_out[:].rearrange("(j p f) -> j p f", j=N_CORES, p=P)

        xt_tiles = {}

        def ensure_xt(i):  # i even: tile pair (i, i+1)
            if i in xt_tiles or i >= NT:
                return
            xt = xt_pool.tile([P, D // P, 2 * P], MMD, tag="xt",
                              name=f"xt{i}")
            nc.sync.dma_start(
                out=xt[:],
                in_=x_t.ap().rearrange("(k p) t -> p k t", p=P)
                    [:, :, i * P:(i + 2) * P])
            xt_tiles[i] = xt

        pending_den = []

        def flush_den_part(step):
            # steps 0-3: reciprocal quarters (short DVE ops so the DVE queue
            # never stalls long); step 4: broadcast + normalize + ship
            if not pending_den:
                return
            ps_y, ps_r, c, rrow = pending_den[0]
            if step < 4:
                q0 = step * (TCH // 4)
                nc.vector.reciprocal(rrow[0:1, q0:q0 + TCH // 4],
                                     ps_r[0:1, q0:q0 + TCH // 4])
                return
            pending_den.pop(0)
            rb_sb = rro_pool.tile([P, TCH], F32, tag="rb_sb")
            nc.gpsimd.partition_broadcast(rb_sb[:], rrow[:])
            nc.vector.tensor_mul(yT_c[c][:], ps_y[:], rb_sb[:])
            nc.gpsimd.dma_start(out=cc_in_v[c], in_=yT_c[c][:])

        def flush_den(n):
            while len(pending_den) > n:
                for st in range(5):
                    flush_den_part(st)

        def qkv_stage(g):
            ensure_xt(4 * g)
            ensure_xt(4 * g + 2)
            ve_g = ve_pool.tile([P, 4, HD], F32, tag="ve", name=f"ve{g}")
            nc.gpsimd.dma_start(
                out=ve_g[:],
                in_=ve_h.ap().rearrange("(n p) e -> p n e", p=P)
                    [:, 4 * g:4 * g + 4, :])
            cos_g = cs_pool.tile([P, 4, QUARTER], MMD, tag="cos",
                                 name=f"cos{g}")
            sin_g = cs_pool.tile([P, 4, QUARTER], MMD, tag="sin",
                                 name=f"sin{g}")
            nc.gpsimd.dma_start(
                out=cos_g[:],
                in_=cos_t.ap().rearrange("(n p) e -> p n e", p=P)
                    [:, 4 * g:4 * g + 4, :])
            nc.gpsimd.dma_start(
                out=sin_g[:],
                in_=sin_t.ap().rearrange("(n p) e -> p n e", p=P)
                    [:, 4 * g:4 * g + 4, :])
            # ssq_g[:, 2i:2i+2] = [sum q^2, sum k^2] for tile 4g+i
            ssq_g = stat_pool.tile([P, 8], F32, tag="ssq", name=f"ssq{g}")
            ps_qkvs = []
            for ii in range(4):
                i = 4 * g + ii
                t0 = i * P
                ensure_xt(i - i % 2)
                xt_huge = xt_tiles[i - i % 2]
                xoff = (i % 2) * P
                ps_qkv = ps.tile([P, 3 * HD], F32, tag="ps",
                                 name=f"psqkv{i}")
                for k in range(D // P):
                    nc.tensor.matmul(ps_qkv[:], xt_huge[:, k, xoff:xoff + P],
                                     wqkv_sb[:, k, :],
                                     start=(k == 0), stop=(k == D // P - 1))
                nc.vector.scalar_tensor_tensor(
                    out=v_t[i][:], in0=ps_qkv[:, 2 * HD:3 * HD],
                    scalar=lam_sb[:, 0:1], in1=ve_g[:, ii, :],
                    op0=mybir.AluOpType.mult, op1=mybir.AluOpType.add)
                qk_sb = qk_pool.tile([P, 2 * HD], F32, tag="qksb",
                                     name=f"qksb{i}")
                if ii % 2:
                    nc.vector.tensor_copy(qk_sb[:], ps_qkv[:, 0:2 * HD])
                else:
                    nc.scalar.copy(qk_sb[:], ps_qkv[:, 0:2 * HD])
                sq = scr_pool.tile([P, HD], F32, tag="sq")
                nc.scalar.activation(sq[:], qk_sb[:, 0:HD],
                                     mybir.ActivationFunctionType.Square,
                                     accum_out=ssq_g[:, 2 * ii:2 * ii + 1])
                nc.scalar.activation(sq[:], qk_sb[:, HD:2 * HD],
                                     mybir.ActivationFunctionType.Square,
                                     accum_out=ssq_g[:, 2 * ii + 1:2 * ii + 2])
                ps_qkvs.append(qk_sb)
            # prefetch next group's x tiles
            ensure_xt(4 * g + 4)
            ensure_xt(4 * g + 6)
            return ssq_g, ps_qkvs, ve_g, cos_g, sin_g

        def norm_stage(g, ssq_g, ps_qkvs, cos_g, sin_g):
            # rsq = 1/sqrt(ssq) batched for the group: integer magic + 2
            # Newton iterations, all on DVE (no ACT table involvement).
            # 1/sqrt(mean) = rsq * sqrt(HD) is folded into the final scales.
            h_i = stat_pool.tile([P, 8], I32, tag="h_i")
            nc.vector.tensor_scalar(
                out=h_i[:], in0=ssq_g[:].bitcast(I32), scalar1=1,
                scalar2=None,
                op0=mybir.AluOpType.logical_shift_right)
            y0 = stat_pool.tile([P, 8], F32, tag="y0")
            nc.vector.tensor_scalar(
                out=y0[:].bitcast(I32), in0=h_i[:], scalar1=-1,
                scalar2=RSQRT_MAGIC,
                op0=mybir.AluOpType.mult, op1=mybir.AluOpType.add)
            t1 = stat_pool.tile([P, 8], F32, tag="t1")
            rsq = stat_pool.tile([P, 8], F32, tag="rsq", name=f"rsq{g}")
            cur = y0
            for it, nxt in ((0, t1), (1, rsq)):
                tt = stat_pool.tile([P, 8], F32, tag=f"tt{it}")
                nc.vector.tensor_mul(tt[:], cur[:], cur[:])
                nc.vector.tensor_mul(tt[:], tt[:], ssq_g[:])
                nc.vector.tensor_scalar(
                    out=tt[:], in0=tt[:], scalar1=-0.5, scalar2=1.5,
                    op0=mybir.AluOpType.mult, op1=mybir.AluOpType.add)
                nc.vector.tensor_mul(nxt[:], cur[:], tt[:])
                cur = nxt

            sq128 = float(np.sqrt(HD))
            for ii in range(4):
                i = 4 * g + ii
                t0 = i * P
                qk_sb = ps_qkvs[ii]
                qkn = tin_pool.tile([P, 2 * HD], MMD, tag="qkn",
                                    name=f"qkn{i}")
                nc.vector.tensor_scalar(
                    out=qkn[:, 0:HD], in0=qk_sb[:, 0:HD],
                    scalar1=rsq[:, 2 * ii:2 * ii + 1],
                    scalar2=ATTN_SCALE * sq128,
                    op0=mybir.AluOpType.mult, op1=mybir.AluOpType.mult)
                nc.vector.tensor_scalar(
                    out=qkn[:, HD:2 * HD], in0=qk_sb[:, HD:2 * HD],
                    scalar1=rsq[:, 2 * ii + 1:2 * ii + 2], scalar2=sq128,
                    op0=mybir.AluOpType.mult, op1=mybir.AluOpType.mult)
                # rope on first-quarter pairs of q AND k in one op each
                def two_rng(tl, col0):
                    src = tl[:]
                    return bass.AP(src.tensor, src.offset + col0,
                                   [list(src.ap[0]), [HD, 2], [1, QUARTER]])

                def cs_b(ap2d):
                    return bass.AP(ap2d.tensor, ap2d.offset,
                                   [list(ap2d.ap[0]), [0, 2],
                                    list(ap2d.ap[-1])])

                x1 = two_rng(qkn, 0)
                x2 = two_rng(qkn, 2 * QUARTER)
                cb_, sb_ = cs_b(cos_g[:, ii, :]), cs_b(sin_g[:, ii, :])
                a = scr_pool.tile([P, 2, QUARTER], MMD, tag="ropeA")
                b = scr_pool.tile([P, 2, QUARTER], MMD, tag="ropeB")
                c2 = scr_pool.tile([P, 2, QUARTER], MMD, tag="ropeC")
                d2 = scr_pool.tile([P, 2, QUARTER], MMD, tag="ropeD")
                nc.vector.tensor_mul(a[:], x1, cb_)
                nc.vector.tensor_mul(b[:], x2, sb_)
                nc.vector.tensor_mul(c2[:], x2, cb_)
                nc.vector.tensor_mul(d2[:], x1, sb_)
                nc.vector.tensor_add(x1, a[:], b[:])
                nc.vector.tensor_sub(x2, c2[:], d2[:])
                # transpose q,k into [e, t] layout (PE transpose, DVE evict)
                sub = ii * P
                for ei, (src_ap, dst, c0) in enumerate(
                        ((qkn[:, 0:HD], qT_c[g], sub),
                         (qkn[:, HD:2 * HD], kT_t[i], 0))):
                    ps_tr = ps.tile([P, P], MMD, tag="ps")
                    nc.tensor.transpose(ps_tr[:], src_ap, ident[:])
                    if ei:
                        nc.vector.tensor_copy(dst[:, c0:c0 + P], ps_tr[:])
                    else:
                        nc.scalar.copy(dst[:, c0:c0 + P], ps_tr[:])

        def attn_chunk(c):
            jmax = 4 * c + 4
            ps_y = psy.tile([P, TCH], F32, tag="psy", name=f"psy{c}")
            acc = acc_pool.tile([P, TCH], MMD, name=f"acc{c}")
            s_psums = {}

            def s_mm(j):
                p_s = ps.tile([P, TCH], F32, tag="ps")
                nc.tensor.matmul(p_s[:], kT_t[j][:], qT_c[c][:],
                                 start=True, stop=True)
                return p_s

            s_psums[0] = s_mm(0)
            for j in range(jmax):
                if j + 1 < jmax:
                    s_psums[j + 1] = s_mm(j + 1)
                if j <= 4:
                    flush_den_part(j)
                p_s = s_psums.pop(j)
                e_sb = exp_pool.tile([P, TCH], MMD)
                nc.scalar.activation(e_sb[:], p_s[:],
                                     mybir.ActivationFunctionType.Exp,
                                     bias=expb_col[:])
                if j >= 4 * c:
                    nc.vector.tensor_mul(e_sb[:], e_sb[:],
                                         masks[j - 4 * c][:])
                if j == 0:
                    nc.vector.tensor_copy(acc[:], e_sb[:])
                else:
                    nc.vector.tensor_add(acc[:], acc[:], e_sb[:])
                nc.tensor.matmul(ps_y[:], v_t[j][:], e_sb[:],
                                 start=(j == 0), stop=(j == jmax - 1))
            ps_r = psy.tile([1, TCH], F32, tag="psy", name=f"psr{c}")
            nc.tensor.matmul(ps_r[:], ones_col[:], acc[:],
                             start=True, stop=True)
            rrow = rro_pool.tile([1, TCH], F32, name=f"rrow{c}")
            pending_den.append((ps_y, ps_r, c, rrow))

        # ---- main loop: qkv(g) | attn(g-1) | norm(g) dovetail ----
        # chunk 0 (4 key-blocks) is processed LAST so the pre-collective
        # tail is as short as possible.
        cpw_tiles = {}
        handles = {0: qkv_stage(0), 1: qkv_stage(1)}
        for g in range(NC_CH):
            if g >= 2:
                attn_chunk(g - 1)
            if g + 2 < NC_CH:
                handles[g + 2] = qkv_stage(g + 2)
            ssq_g, ps_qkvs, ve_g, cos_g, sin_g = handles.pop(g)
            norm_stage(g, ssq_g, ps_qkvs, cos_g, sin_g)
            if g == 5:  # prefetch output-projection weights mid-flight
                for dh in range(D // TCH):
                    for h in range(H):
                        ct = cpw_pool.tile([P, TCH], MMD, tag="cpw",
                                           name=f"cpw{h}_{dh}")
                        nc.gpsimd.dma_start(
                            out=ct[:],
                            in_=cpw.ap()[h * P:(h + 1) * P,
                                         dh * TCH:(dh + 1) * TCH])
                        cpw_tiles[(h, dh)] = ct
        attn_chunk(NC_CH - 1)
        attn_chunk(0)
        flush_den(0)

        # ---- AllToAll: head-parallel -> sequence-parallel ----
        nc.gpsimd.collective_compute(
            "AllToAll", mybir.AluOpType.bypass,
            replica_groups=[list(range(N_CORES))],
            ins=[cc_in[:].opt()], outs=[cc_out[:].opt()])
        yall = [big.tile([P, TCH], MMD, name=f"yall{j}")
                for j in range(N_CORES)]
        for j in range(N_CORES):
            nc.sync.dma_start(out=yall[j][:], in_=cc_out_v[j])

        # ---- output projection for this core's 512 rows ----
        for i in range(SHARD // P):
            for dh in range(D // TCH):
                ps_o = ps.tile([P, TCH], F32, tag="ps")
                for h in range(H):
                    nc.tensor.matmul(ps_o[:],
                                     yall[h][:, i * P:(i + 1) * P],
                                     cpw_tiles[(h, dh)][:],
                                     start=(h == 0), stop=(h == H - 1))
                o_sb = exp_pool.tile([P, TCH], F32, tag="osb")
                nc.scalar.copy(o_sb[:], ps_o[:])
                nc.sync.dma_start(
                    out=y_shard.ap()[i * P:(i + 1) * P,
                                     dh * TCH:(dh + 1) * TCH],
                    in_=o_sb[:])

    nc.compile()
    return nc


def _host_prep(x, ve, qkv_w, lambdas, c_proj_w):
    x = np.asarray(x, dtype=np.float32)
    ve = np.asarray(ve, dtype=np.float32)
    qkv_w = np.asarray(qkv_w, dtype=np.float32)
    lambdas = np.asarray(lambdas, dtype=np.float32)
    c_proj_w = np.asarray(c_proj_w, dtype=np.float32)

    xT = np.ascontiguousarray(x[0].T.astype(NP_MMD))
    cpwT = np.ascontiguousarray(c_proj_w.T.astype(NP_MMD))
    lam_b = np.ascontiguousarray(np.broadcast_to(lambdas, (P, 2)))

    angular = (np.float32(1.0 / 1024.0)
               ** np.linspace(0.0, 1.0, QUARTER, dtype=np.float32))
    t = np.arange(T, dtype=np.float32)
    theta = t[:, None] * angular[None, :]
    cos32 = np.cos(theta).astype(NP_MMD)
    sin32 = np.sin(theta).astype(NP_MMD)

    in_maps = []
    for h in range(N_CORES):
        sl = slice(h * HD, (h + 1) * HD)
        w_qkvT = np.ascontiguousarray(np.concatenate(
            [qkv_w[0, sl, :].T, qkv_w[1, sl, :].T, qkv_w[2, sl, :].T],
            axis=1).astype(NP_MMD))
        in_maps.append({
            "x_t": xT,
            "w_qkv": w_qkvT,
            "cos_t": cos32,
            "sin_t": sin32,
            "ve_h": np.ascontiguousarray(ve[0][:, sl] * lambdas[1]),
            "lam": lam_b,
            "cpw": cpwT,
        })
    return in_maps


def kernel(x, ve, qkv_w, lambdas, c_proj_w, _trace=False, _trace_kwargs=None):
    if "nc" not in _cached:
        _cached["nc"] = build_module()
    nc = _cached["nc"]
    in_maps = _host_prep(x, ve, qkv_w, lambdas, c_proj_w)
    kw = {}
    if _trace:
        kw = dict(trace=True, **(_trace_kwargs or {}))
    res = run_bass_kernel_spmd(nc, in_maps, core_ids=list(range(N_CORES)),
                               **kw)
    _cached["last_result"] = res
    out = np.concatenate([res.results[c]["y_shard"] for c in range(N_CORES)],
                         axis=0)
    return out[None].astype(np.float32)
